# revision 18
# baseline (speedup 1.0000x reference)
"""BinaryNADE Trainium2 kernel (8-core SPMD, h-sharded, d-on-partitions).

Math (reference):
    base = c + W_ctx @ context                      # [H]
    contrib = W_sol * s[None, :]                    # [H, D]
    A = base[:, None] + exclusive_cumsum_d(contrib) # [H, D]
    Hmat = sigmoid(A)                               # [H, D]
    logit[d] = b[d] + sum_h U[d, h] * Hmat[h, d]
    p_dist = sigmoid(logit)
    p_val = prod(p_dist**s + (1 - p_dist[0])**(1 - s))

Sharding: each of the 8 cores owns 512 rows of W (and the matching 512
columns of U); per-core partial dot products are summed on the host, which
also applies b, the final sigmoid, and the p_val reduction (O(D) work).

Per-core layout: d on partitions (64 tiles of 128), h along free (512),
data in fp16 (PSUM accumulates fp32; host fp16 rounding gives ~1e-4 rel err
on p_dist). The exclusive cumsum over d runs entirely on the TensorE:

 - W_sol and s arrive shifted by one d, so shifted-tile row p holds
   contrib[128*dt + p - 1]. An inclusive [128,128] triangular matmul of a
   shifted tile then yields the exclusive prefix within the tile.
 - Per-tile UNSHIFTED block sums accumulate into one PSUM bank via one-hot
   column lhsT matrices (tile dt posts rows 1..127 to column dt and its row
   0 — the previous tile's last element — to column dt-1).
 - One [128,64] triangular matmul turns (block sums, base) into per-tile
   offsets; the offsets round-trip through DRAM into row 0 of every shifted
   contrib tile, so the single inclusive-tri matmul per tile produces
   base + exclusive_cumsum directly.
 - sigmoid on ScalarE (PSUM->SBUF), then one fused DVE scalar_tensor_tensor
   computes Hm*U and its free-axis sum (the per-d partial dot product).
"""

import numpy as np

import concourse.bass as bass
import concourse.bacc as bacc
import concourse.mybir as mybir
from concourse.tile import TileContext
from concourse.bass_utils import run_bass_kernel_spmd

F32 = mybir.dt.float32
F16 = mybir.dt.float16

TRACE = False       # set by test harness to capture an NTFF profile
LAST_RESULT = None

DIM_SOL = 8192      # D
DIM_CONTEXT = 2048  # C
DIM_HIDDEN = 4096   # H
N_CORES = 8
H_SH = DIM_HIDDEN // N_CORES   # 512 hidden rows per core


def build_core_kernel(h_sh=H_SH, c_dim=DIM_CONTEXT, d_dim=DIM_SOL):
    """Per-core Bass program; all cores run it on their own shard."""
    assert d_dim % 128 == 0 and c_dim % 128 == 0
    dt_n = d_dim // 128           # number of d-tiles
    assert dt_n <= 64             # block sums live on <=64 psum partitions
    kt_n = c_dim // 128 + 1       # base k-tiles incl. the c-vector row
    chunk_dt = min(16, dt_n)      # d-tiles per streamed DMA chunk
    n_chunks = dt_n // chunk_dt
    assert dt_n % chunk_dt == 0

    nc = bacc.Bacc("TRN2", target_bir_lowering=False, debug=False)

    # tiled [128, dt_n*h_sh] fp16: (p, dt*h_sh + h) = X[128*dt + p, h]
    wst = nc.dram_tensor("wst", [128, dt_n * h_sh], F16, kind="ExternalInput")
    utt = nc.dram_tensor("utt", [128, dt_n * h_sh], F16, kind="ExternalInput")
    wctt = nc.dram_tensor("wctt", [128, kt_n * h_sh], F16, kind="ExternalInput")
    ctxc = nc.dram_tensor("ctxc", [128, kt_n], F16, kind="ExternalInput")
    scol = nc.dram_tensor("scol", [128, dt_n], F32, kind="ExternalInput")
    emat = nc.dram_tensor("emat", [128, dt_n * dt_n], F16, kind="ExternalInput")
    trit = nc.dram_tensor("trit", [128, 128], F16, kind="ExternalInput")
    stri = nc.dram_tensor("stri", [128, dt_n], F16, kind="ExternalInput")
    outp = nc.dram_tensor("outp", [128, dt_n], F32, kind="ExternalOutput")
    offsd = nc.dram_tensor("offsd", [dt_n, h_sh], F16, kind="Internal")

    with TileContext(nc) as tc:
        with (
            tc.tile_pool(name="const", bufs=1) as constp,
            tc.tile_pool(name="wstp", bufs=2) as wstp,
            tc.tile_pool(name="uttp", bufs=2) as uttp,
            tc.tile_pool(name="hmp", bufs=4) as hmp,
            tc.tile_pool(name="vscrp", bufs=3) as vscrp,
            tc.tile_pool(name="psA", bufs=1, space="PSUM") as psap,
            tc.tile_pool(name="psmisc", bufs=1, space="PSUM") as psmp,
        ):
            # ---- constants (order = DMA issue order) -----------------------
            scol_sb = constp.tile([128, dt_n], F32)
            nc.sync.dma_start(out=scol_sb[:, :], in_=scol[:, :])
            em_sb = constp.tile([128, dt_n * dt_n], F16)
            nc.sync.dma_start(out=em_sb[:, :], in_=emat[:, :])
            tri_sb = constp.tile([128, 128], F16)
            nc.sync.dma_start(out=tri_sb[:, :], in_=trit[:, :])
            stri_sb = constp.tile([128, dt_n], F16)
            nc.sync.dma_start(out=stri_sb[:, :], in_=stri[:, :])
            ctx_sb = constp.tile([128, kt_n], F16)
            nc.sync.dma_start(out=ctx_sb[:, :], in_=ctxc[:, :])

            contrib = constp.tile([128, dt_n * h_sh], F16)
            totals_sb = constp.tile([128, h_sh], F16)
            nc.vector.memset(totals_sb[:, :], 0.0)
            offs16 = constp.tile([dt_n, h_sh], F16)
            out_sb = constp.tile([128, dt_n], F32)

            # ---- phase 1: shifted contrib tiles + unshifted block sums -----
            ps_tot = psmp.tile([dt_n, h_sh], F32)
            for ch in range(n_chunks):
                wst_t = wstp.tile([128, chunk_dt * h_sh], F16)
                nc.sync.dma_start(
                    out=wst_t[:, :],
                    in_=wst[:, chunk_dt * h_sh * ch:chunk_dt * h_sh * (ch + 1)],
                )
                for j in range(chunk_dt):
                    dt = ch * chunk_dt + j
                    nc.vector.tensor_scalar_mul(
                        contrib[:, h_sh * dt:h_sh * (dt + 1)],
                        wst_t[:, h_sh * j:h_sh * (j + 1)],
                        scol_sb[:, dt:dt + 1],
                    )
                    nc.tensor.matmul(
                        ps_tot[:, :],
                        em_sb[:, dt_n * dt:dt_n * (dt + 1)],
                        contrib[:, h_sh * dt:h_sh * (dt + 1)],
                        start=(dt == 0), stop=(dt == dt_n - 1),
                    )

            # ---- base row (overlaps phase 1): ctx^T @ W_ctx^T + c ----------
            wct_sb = constp.tile([128, kt_n * h_sh], F16)
            nc.sync.dma_start(out=wct_sb[:, :], in_=wctt[:, :])
            ps_base = psmp.tile([1, h_sh], F32)
            for kt in range(kt_n):
                nc.tensor.matmul(
                    ps_base[:, :],
                    ctx_sb[:, kt:kt + 1],
                    wct_sb[:, h_sh * kt:h_sh * (kt + 1)],
                    start=(kt == 0), stop=(kt == kt_n - 1),
                )
            nc.scalar.copy(totals_sb[64:65, :], ps_base[:, :])
            nc.scalar.copy(totals_sb[0:dt_n, :], ps_tot[:, :])

            # ---- phase 2: offsets -> DRAM -> row 0 of each contrib tile ----
            ps_offs = psmp.tile([dt_n, h_sh], F32)
            nc.tensor.matmul(ps_offs[:, :], stri_sb[:, :], totals_sb[:, :],
                             start=True, stop=True)
            nc.scalar.copy(offs16[:, :], ps_offs[:, :])
            nc.sync.dma_start(out=offsd[:, :], in_=offs16[:, :])
            nc.sync.dma_start(
                out=contrib[0:1, :],
                in_=offsd.ap().rearrange("a b -> (a b)")[None, :],
            )

            # ---- phase 3: A tiles, sigmoid, fused dot ----------------------
            for ch in range(n_chunks):
                utt_t = uttp.tile([128, chunk_dt * h_sh], F16)
                nc.sync.dma_start(
                    out=utt_t[:, :],
                    in_=utt[:, chunk_dt * h_sh * ch:chunk_dt * h_sh * (ch + 1)],
                )
                for j0 in range(0, chunk_dt, 4):
                    grp = range(j0, min(j0 + 4, chunk_dt))
                    ps_as = {}
                    for j in grp:
                        dt = ch * chunk_dt + j
                        ps_a = psap.tile([128, h_sh], F32, name=f"ps_a{j % 4}",
                                         tag=f"ps_a{j % 4}")
                        ps_as[j] = ps_a
                        nc.tensor.matmul(ps_a[:, :], tri_sb[:, :],
                                         contrib[:, h_sh * dt:h_sh * (dt + 1)],
                                         start=True, stop=True)
                    for j in grp:
                        dt = ch * chunk_dt + j
                        hm_t = hmp.tile([128, h_sh], F16)
                        nc.scalar.activation(hm_t[:, :], ps_as[j][:, :],
                                             mybir.ActivationFunctionType.Sigmoid)
                        vscr = vscrp.tile([128, h_sh], F16)
                        nc.vector.scalar_tensor_tensor(
                            vscr[:, :], hm_t[:, :], 1.0,
                            utt_t[:, h_sh * j:h_sh * (j + 1)],
                            mybir.AluOpType.mult, mybir.AluOpType.mult,
                            accum_out=out_sb[:, dt:dt + 1])

            nc.sync.dma_start(out=outp[:, :], in_=out_sb[:, :])

    nc.compile()
    return nc


def make_in_maps(context, solution, W, U, c,
                 h_sh=H_SH, c_dim=DIM_CONTEXT, d_dim=DIM_SOL, n_cores=N_CORES):
    """Host-side shard/layout prep. Layout + dtype only — no model math."""
    f16 = np.float16
    dt_n = d_dim // 128
    kt_n = c_dim // 128 + 1

    ctxa = np.zeros(kt_n * 128, np.float32)
    ctxa[:c_dim] = context
    ctxa[c_dim] = 1.0
    ctxc = np.ascontiguousarray(ctxa.reshape(kt_n, 128).T).astype(f16)

    # s shifted by one d (s[-1] := 0), per-partition column layout
    s_sh = np.concatenate([[np.float32(0.0)], solution[:-1]]).astype(np.float32)
    scol = np.ascontiguousarray(s_sh.reshape(dt_n, 128).T)

    # emat[:, dt-block]: tile dt posts rows 1..127 to column dt and row 0
    # (prev tile's last element) to column dt-1 -> unshifted block sums.
    emat = np.zeros((128, dt_n * dt_n), f16)
    for dt in range(dt_n):
        emat[1:, dt_n * dt + dt] = 1.0
        if dt > 0:
            emat[0, dt_n * dt + dt - 1] = 1.0

    trit = np.tri(128, 128, 0, dtype=f16).T          # [p, i] = 1 if p <= i
    # stri[p, t] = 1 if p < t (strict prefix over block sums) or p == 64 (base)
    stri = np.triu(np.ones((128, dt_n), f16), 1)
    stri[64:, :] = 0.0
    stri[64, :] = 1.0

    def tile_pd(x):  # [d_dim, h_sh] -> [128, dt_n*h_sh] fp16 tiled layout
        return np.ascontiguousarray(
            x.reshape(dt_n, 128, h_sh).transpose(1, 0, 2).reshape(
                128, dt_n * h_sh)).astype(f16)

    in_maps = []
    for core in range(n_cores):
        h0 = core * h_sh
        wsol = np.ascontiguousarray(W[h0:h0 + h_sh, c_dim:].T)  # [d, h]
        wsol_sh = np.vstack([np.zeros((1, h_sh), np.float32), wsol[:-1]])
        wst = tile_pd(wsol_sh)
        utt = tile_pd(np.ascontiguousarray(U[:, h0:h0 + h_sh]))
        wcta = np.zeros((kt_n * 128, h_sh), np.float32)
        wcta[:c_dim] = W[h0:h0 + h_sh, :c_dim].T
        wcta[c_dim] = c[h0:h0 + h_sh]
        wctt = np.ascontiguousarray(
            wcta.reshape(kt_n, 128, h_sh).transpose(1, 0, 2).reshape(
                128, kt_n * h_sh)).astype(f16)
        in_maps.append(dict(wst=wst, utt=utt, wctt=wctt, ctxc=ctxc,
                            scol=scol, emat=emat, trit=trit, stri=stri))
    return in_maps


def kernel(context, solution, W, U, b, c):
    context = np.ascontiguousarray(np.asarray(context, np.float32))
    solution = np.ascontiguousarray(np.asarray(solution, np.float32))
    W = np.ascontiguousarray(np.asarray(W, np.float32))
    U = np.ascontiguousarray(np.asarray(U, np.float32))
    b = np.ascontiguousarray(np.asarray(b, np.float32))
    c = np.ascontiguousarray(np.asarray(c, np.float32))

    nc = build_core_kernel()
    in_maps = make_in_maps(context, solution, W, U, c)
    res = run_bass_kernel_spmd(nc, in_maps, core_ids=list(range(N_CORES)),
                               trace=TRACE)
    global LAST_RESULT
    LAST_RESULT = res

    partial = np.zeros(DIM_SOL, np.float32)
    for r in res.results:
        partial += r["outp"].T.reshape(DIM_SOL)  # d = 128*dt + p

    logits = (b + partial).astype(np.float32)
    p_dist = (1.0 / (1.0 + np.exp(-logits, dtype=np.float32))).astype(np.float32)
    terms = (np.power(p_dist, solution) +
             np.power(np.float32(1.0) - p_dist[0],
                      np.float32(1.0) - solution)).astype(np.float32)
    p_val = np.prod(terms, dtype=np.float32)
    return (np.float32(p_val), p_dist)
